# revision 26
# baseline (speedup 1.0000x reference)
"""Multi-head causal attention with RoPE on 8 Trainium2 NeuronCores.

Sharding: data-parallel over batch (B=2) x tensor-parallel over heads
(16 heads -> 4 groups of 4). Core c handles batch c//4, heads
[(c%4)*4, (c%4)*4+4). Each core computes a partial y = attn_out @ W_o
for its head group; the host sums the 4 partials per batch (the "W_o
all-reduce"). Partials ship back in bf16 (halves output DMA).

Schedule (v2): one continuous pipeline instead of serial phases.
  - Input DMA striped per E-chunk in consumption order (wk_ec, wq_ec,
    x_ec trios alternating across both HWDGE rings) so the first
    projection matmul starts ~1us in and the PE stays busy through the
    DMA window (keeps the HAM clock gate warm from the start).
  - K/Q projections for mb=0 run chunk-driven (i-outer) into 4 PSUM
    tiles (8 banks); drains split across ACT and DVE.
  - V is computed directly in natural layout (t on partitions) via
    lhsT = x^T t-block slices -- kills the 32 PE transposes + copies.
  - RoPE: same folded-permutation trick as v1 (host permutes W_q/W_k
    columns so the rotation partner is +-16 partitions away; one DVE
    stream_shuffle + cos/sin multiply-adds).
  - Attention runs heads SEQUENTIALLY (not pair-interleaved): sc pool
    2x[128,1024] (4 banks) + one acc [65,1024] (2 banks) leaves 2 PSUM
    banks free, into which the mb=1 K/Q projection matmuls are
    INJECTED during h0/h1's ACT-bound slack (one small thunk per
    t-block iteration, issued between the score matmuls and the
    previous iteration's PV so the PE never waits on ScalarE).
    rope_k(1) combine runs on GpSimd, rope_q(1) ops inject on DVE.
  - Scores/PV per head: scores^T[t,q] = K^T.T @ Qz (zero-padded K=128
    streams -- HAM never grants full clock to K=64), exp on ACT
    (scale folded), causal mask mul on the diagonal block on DVE, PV
    software-pipelined one iteration behind, denominators via a ones
    column in V. 2-pass reciprocal + gpsimd partition_broadcast
    normalize, deferred one pass so PSUM-release sems fire early.
  - Phase C (y = onrm^T.T @ W_o) at the end, bf16 output, DMA per
    half-block alternating rings.
"""

import os
import sys
from collections import deque
from contextlib import ExitStack

import numpy as np

for _p in ("/opt/trn_rl_repo",):
    if os.path.isdir(_p) and _p not in sys.path:
        sys.path.insert(0, _p)

import ml_dtypes  # noqa: E402

BF16 = ml_dtypes.bfloat16

B, S, E = 2, 2048, 1024
H, DH = 16, 64
NCORES = 8
HPC = H // 4          # 4 heads per core
DC = HPC * DH         # 256 head dims per core
ATTN_SCALE = 1.0 / 32.0  # 1/sqrt(E)
ROPE_BASE = 10000.0
P = 128
NSB = S // P          # 16 sequence blocks
NEC = E // P          # 8 E chunks
MB = DC // P          # 2 partition blocks of head dims

_PROG = None


def _perm64():
    """perm[j] = original head-dim index stored at permuted position j.

    Quadrant q of the permuted layout holds RoPE pairs i in
    [16q, 16q+16): even elements (2i) at slots 0-15, odd (2i+1) at
    slots 16-31. The rotation partner is then always +-16 partitions
    away within one 32-partition quadrant (stream_shuffle range).
    """
    j = np.arange(64)
    qd, r = j // 32, j % 32
    i = 16 * qd + (r % 16)
    return 2 * i + (r >= 16)


def _cos_sin_tiles():
    pl = np.arange(P) % 64
    qd, r = pl // 32, pl % 32
    i = 16 * qd + (r % 16)
    inv = ROPE_BASE ** (-(2.0 * i) / DH)
    ang = np.arange(S)[None, :] * inv[:, None]          # (128, S)
    sgn = np.where(r < 16, -1.0, 1.0)[:, None]
    return ang, sgn


def _build_program():
    import concourse.bacc as bacc
    import concourse.tile as tile
    from concourse import mybir

    f32 = mybir.dt.float32
    bf16 = mybir.dt.bfloat16
    AF = mybir.ActivationFunctionType

    nc = bacc.Bacc("TRN2", target_bir_lowering=False, debug=False)
    xbt = nc.dram_tensor("xbt", [E, S], bf16, kind="ExternalInput").ap()
    wq = nc.dram_tensor("wq", [E, DC], bf16, kind="ExternalInput").ap()
    wk = nc.dram_tensor("wk", [E, DC], bf16, kind="ExternalInput").ap()
    wv = nc.dram_tensor("wv", [E, DC], bf16, kind="ExternalInput").ap()
    wo = nc.dram_tensor("wo", [DC, E], bf16, kind="ExternalInput").ap()
    cosr = nc.dram_tensor("cosr", [P, S], bf16, kind="ExternalInput").ap()
    sinr = nc.dram_tensor("sinr", [P, S], bf16, kind="ExternalInput").ap()
    cmask = nc.dram_tensor("cmask", [P, P], bf16, kind="ExternalInput").ap()
    y = nc.dram_tensor("y", [S, E], bf16, kind="ExternalOutput").ap()

    wkr = wk.rearrange("(c p) m -> p c m", p=P)
    wqr = wq.rearrange("(c p) m -> p c m", p=P)
    wvr = wv.rearrange("(c p) m -> p c m", p=P)

    with ExitStack() as ctx:
        tc = ctx.enter_context(tile.TileContext(nc))
        consts = ctx.enter_context(tc.tile_pool(name="consts", bufs=1))
        persist = ctx.enter_context(tc.tile_pool(name="persist", bufs=1))

        kcT = persist.tile([P, MB, S], bf16, tag="kcT")
        qcT = persist.tile([P, MB, S], bf16, tag="qcT")
        # qz holds RoPE'd Q^T zero-padded per head parity: slice
        # [:, mb, par, :] has head (2*mb+par)'s 64 rows live and the
        # other 64 rows zero (scores then run full K=128 streams).
        qz = persist.tile([P, MB, 2, S], bf16, tag="qz")
        kT = persist.tile([P, MB, S], bf16, tag="kT")
        vn = persist.tile([P, NSB, HPC, 65], bf16, tag="vn")
        onrm = persist.tile([P, MB, S], bf16, tag="onrm")
        xT = persist.tile([P, NEC, S], bf16, tag="xT")

        wk_t = consts.tile([P, NEC, DC], bf16, tag="wk")
        wq_t = consts.tile([P, NEC, DC], bf16, tag="wq")
        wv_t = consts.tile([P, NEC, DC], bf16, tag="wv")
        cos_t = consts.tile([P, S], bf16, tag="cos")
        sin_t = consts.tile([P, S], bf16, tag="sin")
        wo_t = consts.tile([P, MB, E], bf16, tag="wo")
        msk_t = consts.tile([P, P], bf16, tag="msk")

        # ---- input DMA ----
        # Per-chunk (wk, wq, x) trios striped across the two HWDGE
        # rings in consumption order: the chunk-driven projection
        # starts on the first trio (~12us after the DMA preamble) and
        # streams at the 2-ring HBM-limited cadence. wv rides the sync
        # ring just after the x6 trio so V can start right after the
        # projections; cos/sin arrive in time for the RoPE chain.
        for ec in range(NEC):
            eng = nc.sync if ec % 2 == 0 else nc.scalar
            eng.dma_start(wk_t[:, ec, :], wkr[:, ec, :])
            eng.dma_start(wq_t[:, ec, :], wqr[:, ec, :])
            eng.dma_start(xT[:, ec, :], xbt[ec * P:(ec + 1) * P, :])
            if ec == 6:
                nc.sync.dma_start(wv_t[:], wvr)
        nc.scalar.dma_start(cos_t[:], cosr)
        nc.sync.dma_start(sin_t[:], sinr)
        nc.sync.dma_start(msk_t[:], cmask)
        nc.scalar.dma_start(wo_t[:], wo.rearrange("(c p) n -> p c n", p=P))

        shuf_mask = list(range(16, 32)) + list(range(16))
        nc.gpsimd.memset(qz[0:DH, :, 1, :], 0.0)
        nc.gpsimd.memset(qz[DH:P, :, 0, :], 0.0)
        nc.gpsimd.memset(vn[:, :, :, 64:65], 1.0)

        def rope_k_ops(mb, pool):
            # 4 separate DVE thunks so mb=1 can inject them one per
            # iteration without a long head-of-line block on the DVE
            # FIFO (a GpSimd combine here proved disastrous: the
            # epilogue's partition_broadcast queued behind ~13us of
            # GpSimd TTs, stalling the acc-freeing copies).
            sh = pool.tile([P, S], bf16, tag="shuf", name=f"shk{mb}")
            return [
                lambda: nc.vector.stream_shuffle(
                    sh[:], kcT[:, mb, :], shuf_mask
                ),
                lambda: nc.vector.tensor_mul(sh[:], sh[:], sin_t[:]),
                lambda: nc.vector.tensor_mul(
                    kT[:, mb, :], kcT[:, mb, :], cos_t[:]
                ),
                lambda: nc.vector.tensor_add(
                    kT[:, mb, :], kT[:, mb, :], sh[:]
                ),
            ]

        def rope_q_ops(mb, pool):
            # returns the 6 DVE ops as thunks (so mb=1 can inject them)
            sh = pool.tile([P, S], bf16, tag="shuf", name=f"shq{mb}")
            ops = [
                lambda: nc.vector.stream_shuffle(
                    sh[:], qcT[:, mb, :], shuf_mask
                ),
                lambda: nc.vector.tensor_mul(sh[:], sh[:], sin_t[:]),
            ]
            for par in range(2):
                o0 = par * DH

                def mk(par=par, o0=o0):
                    return [
                        lambda: nc.vector.tensor_mul(
                            qz[o0:o0 + DH, mb, par, :],
                            qcT[o0:o0 + DH, mb, :],
                            cos_t[o0:o0 + DH, :],
                        ),
                        lambda: nc.vector.tensor_add(
                            qz[o0:o0 + DH, mb, par, :],
                            qz[o0:o0 + DH, mb, par, :],
                            sh[o0:o0 + DH, :],
                        ),
                    ]

                ops.extend(mk())
            return ops

        # ---- Phase A: K0/Q0 chunk-driven, V natural, rope mb=0 ----
        with ExitStack() as actx:
            pr_ps = actx.enter_context(
                tc.tile_pool(name="pr_ps", bufs=4, space="PSUM")
            )
            rtmp = actx.enter_context(tc.tile_pool(name="rtmp", bufs=2))

            pk = [pr_ps.tile([P, S // 2], f32, tag="prj", name=f"pk{h}")
                  for h in range(2)]
            pq = [pr_ps.tile([P, S // 2], f32, tag="prj", name=f"pq{h}")
                  for h in range(2)]
            for ec in range(NEC):
                for wt, ps in ((wk_t, pk), (wq_t, pq)):
                    for half in range(2):
                        for qt in range(2):
                            nc.tensor.matmul(
                                ps[half][:, qt * 512:(qt + 1) * 512],
                                lhsT=wt[:, ec, 0:P],
                                rhs=xT[:, ec,
                                       half * 1024 + qt * 512:
                                       half * 1024 + qt * 512 + 512],
                                start=(ec == 0),
                                stop=(ec == NEC - 1),
                            )
            # drains split ACT/DVE so neither serializes
            nc.scalar.copy(kcT[:, 0, 0:1024], pk[0][:])
            nc.vector.tensor_copy(kcT[:, 0, 1024:2048], pk[1][:])
            nc.scalar.copy(qcT[:, 0, 0:1024], pq[0][:])
            nc.vector.tensor_copy(qcT[:, 0, 1024:2048], pq[1][:])

            for op in rope_k_ops(0, rtmp):
                op()
            for op in rope_q_ops(0, rtmp):
                op()

        with ExitStack() as vctx:
            v_ps = vctx.enter_context(
                tc.tile_pool(name="v_ps", bufs=6, space="PSUM")
            )
            for sb in range(NSB):
                ps = v_ps.tile([P, DC], f32, tag="vp")
                for ec in range(NEC):
                    nc.tensor.matmul(
                        ps[:],
                        lhsT=xT[:, ec, sb * P:(sb + 1) * P],
                        rhs=wv_t[:, ec, :],
                        start=(ec == 0),
                        stop=(ec == NEC - 1),
                    )
                # ACT copy: DVE is busy with RoPE here, and v_ps bank
                # recycling must not wait behind the RoPE FIFO
                nc.scalar.copy(
                    vn[:, sb, :, 0:64],
                    ps[:].rearrange("p (a b) -> p a b", a=HPC),
                )

        # ---- Phase B: attention, heads sequential, injection ----
        with ExitStack() as bctx:
            sc_ps = bctx.enter_context(
                tc.tile_pool(name="sc_ps", bufs=2, space="PSUM")
            )
            ac_ps = bctx.enter_context(
                tc.tile_pool(name="ac_ps", bufs=1, space="PSUM")
            )
            inj_ps = bctx.enter_context(
                tc.tile_pool(name="inj_ps", bufs=1, space="PSUM")
            )
            ptp = bctx.enter_context(tc.tile_pool(name="ptp", bufs=4))
            dn = bctx.enter_context(tc.tile_pool(name="dn", bufs=2))
            yob = bctx.enter_context(tc.tile_pool(name="yob", bufs=2))

            # --- injected mb=1 projection thunks ---
            # Each PE thunk = 2 matmuls (one qt pair, shared LDWEIGHTS,
            # ~0.43us) into the 2 spare PSUM banks. Copy thunks drain
            # to SBUF on DVE/ACT. rope_k(1) combine goes to GpSimd.
            inj = deque()

            def proj1_thunks(wt, dstT, lbl):
                for half in range(2):
                    pt_ = inj_ps.tile([P, S // 2], f32, tag="inj",
                                      name=f"inj_{lbl}_{half}")
                    for ec in range(NEC):
                        def mm(pt_=pt_, ec=ec, half=half, wt=wt):
                            for qt in range(2):
                                nc.tensor.matmul(
                                    pt_[:, qt * 512:(qt + 1) * 512],
                                    lhsT=wt[:, ec, P:2 * P],
                                    rhs=xT[:, ec,
                                           half * 1024 + qt * 512:
                                           half * 1024 + qt * 512 + 512],
                                    start=(ec == 0),
                                    stop=(ec == NEC - 1),
                                )
                        inj.append(mm)

                    def cp(pt_=pt_, half=half, dstT=dstT):
                        eng = nc.vector.tensor_copy if half == 0 \
                            else nc.scalar.copy
                        eng(dstT[:, 1, half * 1024:(half + 1) * 1024],
                            pt_[:])
                    inj.append(cp)

            proj1_thunks(wk_t, kcT, "k1")
            for op in rope_k_ops(1, dn):
                inj.append(op)
            proj1_thunks(wq_t, qcT, "q1")
            for op in rope_q_ops(1, dn):
                inj.append(op)

            deferred_norms = []

            # phase C for q-blocks 0..7 injects into h3/pss1 (the only
            # pass with free PE slack and complete onrm cols [0,1024):
            # all heads' pss0 chunks normalize by its iter 2). Uses the
            # inj_ps banks, idle once the mb=1 projections drained.
            cinj = deque()

            def phasec_thunks(sb_i):
                yp = inj_ps.tile([P, E], f32, tag="inj", name=f"cyp{sb_i}")

                def mm():
                    for mb in range(MB):
                        for half in range(2):
                            nc.tensor.matmul(
                                yp[:, half * 512:(half + 1) * 512],
                                lhsT=onrm[:, mb, sb_i * P:(sb_i + 1) * P],
                                rhs=wo_t[:, mb, half * 512:(half + 1) * 512],
                                start=(mb == 0),
                                stop=(mb == MB - 1),
                            )
                cinj.append(mm)

                def cp():
                    # DMA on sync only: a dma_start on the scalar ring
                    # here would sit in the exp stream's queue
                    ys = yob.tile([P, E], bf16, tag="ys", name=f"cys{sb_i}")
                    nc.vector.tensor_copy(ys[:, 0:512], yp[:, 0:512])
                    nc.scalar.copy(ys[:, 512:1024], yp[:, 512:1024])
                    nc.sync.dma_start(y[sb_i * P:(sb_i + 1) * P, :], ys[:])
                cinj.append(cp)

            def epilogue_copies(h, acc, q0, c0, c1):
                # stage out^T + the denominator row out of PSUM (plain
                # DVE copies); these two copies are all that holds the
                # accumulator banks.
                w = c1 - c0
                l0 = c0 - q0
                acb = dn.tile([DH, w], f32, tag="acb", name=f"acb{h}_{c0}")
                nc.vector.tensor_copy(acb[:], acc[0:DH, l0:l0 + w])
                den0 = dn.tile([1, w], f32, tag="den0", name=f"den{h}_{c0}")
                nc.vector.tensor_copy(den0[:], acc[64:65, l0:l0 + w])
                return h, c0, c1, acb, den0

            def epilogue_norm(h, c0, c1, acb, den0):
                # 2-pass approximate reciprocal (~22 bits), a partition
                # broadcast on GpSimd (reads partition 0 -> base-0
                # tile), then one multiply.
                mb, off = h // 2, (h % 2) * DH
                w = c1 - c0
                rden = dn.tile([1, w], f32, tag="rden", name=f"rden{h}_{c0}")
                rscr = dn.tile([1, w], f32, tag="rscr", name=f"rscr{h}_{c0}")
                nc.vector.reciprocal_approx_accurate(rden[:], den0[:], rscr[:])
                rdb = dn.tile([DH, w], f32, tag="rdb", name=f"rdb{h}_{c0}")
                nc.gpsimd.partition_broadcast(rdb[:], rden[:])
                nc.vector.tensor_mul(
                    onrm[off:off + DH, mb, c0:c1], acb[:], rdb[:]
                )

            for h in range(HPC):
                mb, par = h // 2, h % 2
                for pss in range(2):
                    q0 = pss * 1024
                    acc = ac_ps.tile([65, 1024], f32, tag="acc",
                                     name=f"acc_{h}_{pss}")

                    def issue_pv(ti, pt, lo, hi, acc=acc, q0=q0, h=h):
                        # one PV piece per PSUM bank; bank bk (global)
                        # is complete at ti == 4*bk+3
                        p0 = lo
                        while p0 < hi:
                            bk = p0 // 512
                            p1 = min(hi, (bk + 1) * 512)
                            nc.tensor.matmul(
                                acc[:, p0 - q0:p1 - q0],
                                lhsT=vn[:, ti, h, :],
                                rhs=pt[:, p0 - q0:p1 - q0],
                                start=(ti == 0),
                                stop=(ti == 4 * bk + 3),
                            )
                            p0 = p1

                    # PV runs TWO iterations behind the scores (lag-2):
                    # by the time PV(ti) is issued, exp(ti) finished
                    # ~1 iteration ago, so the PE never waits on ACT --
                    # and a pass's first PV lands at iter 2, giving the
                    # previous pass's PSUM-freeing copies a 2-iter
                    # window to drain before the acc banks are reused.
                    pending = deque()
                    for ti in range(8 if pss == 0 else NSB):
                        if ti == 2 and deferred_norms:
                            # previous pass's normalize chains, emitted
                            # here so the PSUM-release semaphores are
                            # not queued behind reciprocal work
                            for st in deferred_norms:
                                epilogue_norm(*st)
                            deferred_norms = []
                        if pss == 1 and ti == 14:
                            # PSUM bank 2 got its last PV (ti==11) at
                            # iter 13: free it now (copies only -- the
                            # reciprocal chain is deferred so nothing
                            # here waits on GpSimd)
                            deferred_norms.append(
                                epilogue_copies(h, acc, q0, q0, q0 + 512)
                            )
                        if h == 3 and pss == 1 and ti >= 5 and cinj:
                            cinj.popleft()()
                        t0 = ti * P
                        lo = max(t0, q0)
                        hi = q0 + 1024
                        sc = sc_ps.tile([P, 1024], f32, tag="sc",
                                        name=f"sc_{h}_{pss}_{ti}")
                        p0 = lo
                        while p0 < hi:
                            p1 = min(hi, (p0 // 512 + 1) * 512)
                            nc.tensor.matmul(
                                sc[:, p0 - q0:p1 - q0],
                                lhsT=kT[:, mb, t0:t0 + P],
                                rhs=qz[:, mb, par, p0:p1],
                            )
                            p0 = p1
                        # injected PE work goes here: it runs while ACT
                        # computes the exp the next PV is waiting on
                        if inj:
                            inj.popleft()()
                        pt = ptp.tile([P, 1024], bf16, tag="pt")
                        nc.scalar.activation(
                            pt[:, lo - q0:hi - q0],
                            sc[:, lo - q0:hi - q0],
                            AF.Exp,
                            scale=ATTN_SCALE,
                        )
                        if t0 >= q0:
                            nc.vector.tensor_mul(
                                pt[:, t0 - q0:t0 - q0 + P],
                                pt[:, t0 - q0:t0 - q0 + P],
                                msk_t[:],
                            )
                        pending.append((ti, pt, lo, hi))
                        if len(pending) > 2:
                            issue_pv(*pending.popleft())
                    while pending:
                        issue_pv(*pending.popleft())
                    # pass boundary: ACT lags a few exps behind here,
                    # so the PE would idle waiting for the sc ring --
                    # feed it a couple of extra injected thunks instead
                    for _ in range(3):
                        if inj:
                            inj.popleft()()

                    if pss == 1:
                        chunks = [(h, acc, q0, q0 + 512, q0 + 1024)]
                    else:
                        chunks = [(h, acc, q0, q0, q0 + 1024)]
                    staged = [epilogue_copies(*c) for c in chunks]
                    if h == HPC - 1 and pss == 1:
                        for st in deferred_norms + staged:
                            epilogue_norm(*st)
                        deferred_norms = []
                    else:
                        deferred_norms = deferred_norms + staged
                    # build the phase-C-lo thunks once cols [0,1024)
                    # of onrm are fully staged (h3/pss0 done)
                    if h == 3 and pss == 0:
                        for sb_i in range(8):
                            phasec_thunks(sb_i)
                # all mb=1 work must be in before h2 starts
                if h == 1:
                    while inj:
                        inj.popleft()()
            # any phase-C-lo leftovers
            while cinj:
                cinj.popleft()()

        # ---- Phase C: output projection ----
        with ExitStack() as cctx:
            y_ps = cctx.enter_context(
                tc.tile_pool(name="y_ps", bufs=2, space="PSUM")
            )
            yo = cctx.enter_context(tc.tile_pool(name="yo", bufs=3))
            for sb_i in range(8, NSB):
                yp = y_ps.tile([P, E], f32, tag="yp")
                for mb in range(MB):
                    for half in range(2):
                        nc.tensor.matmul(
                            yp[:, half * 512:(half + 1) * 512],
                            lhsT=onrm[:, mb, sb_i * P:(sb_i + 1) * P],
                            rhs=wo_t[:, mb, half * 512:(half + 1) * 512],
                            start=(mb == 0),
                            stop=(mb == MB - 1),
                        )
                ys = yo.tile([P, E], bf16, tag="ys")
                for half in range(2):
                    sl = slice(half * 512, (half + 1) * 512)
                    if (sb_i + half) % 2 == 0:
                        nc.vector.tensor_copy(ys[:, sl], yp[:, sl])
                    else:
                        nc.scalar.copy(ys[:, sl], yp[:, sl])
                # alternate rings: 2MB on one 180GB/s queue would be
                # an 11us serial tail
                eng = nc.sync if sb_i % 2 == 0 else nc.scalar
                eng.dma_start(y[sb_i * P:(sb_i + 1) * P, :], ys[:])

    nc.compile()
    return nc


def get_program():
    global _PROG
    if _PROG is None:
        _PROG = _build_program()
    return _PROG


def make_in_maps(x, W_q, W_k, W_v, W_o):
    perm = _perm64()
    idx_local = (np.arange(DC) // 64) * 64 + perm[np.arange(DC) % 64]
    ang, sgn = _cos_sin_tiles()
    cos_np = np.cos(ang).astype(BF16)
    sin_np = (sgn * np.sin(ang)).astype(BF16)
    # scores tile is (t, q): keep t <= q -> upper triangular incl. diagonal
    cmask_np = np.triu(np.ones((P, P))).astype(BF16)
    in_maps = []
    for c in range(NCORES):
        b, hg = c // 4, c % 4
        base = hg * DC
        in_maps.append(
            dict(
                xbt=np.ascontiguousarray(x[b].T.astype(BF16)),
                wq=np.ascontiguousarray(W_q[:, base + idx_local].astype(BF16)),
                wk=np.ascontiguousarray(W_k[:, base + idx_local].astype(BF16)),
                wv=np.ascontiguousarray(W_v[:, base:base + DC].astype(BF16)),
                wo=np.ascontiguousarray(W_o[base:base + DC, :].astype(BF16)),
                cosr=cos_np,
                sinr=sin_np,
                cmask=cmask_np,
            )
        )
    return in_maps


def kernel(x, W_q, W_k, W_v, W_o, _trace=False, _trace_cores=None):
    from concourse.bass_utils import run_bass_kernel_spmd

    x = np.asarray(x, dtype=np.float32)
    W_q = np.asarray(W_q, dtype=np.float32)
    W_k = np.asarray(W_k, dtype=np.float32)
    W_v = np.asarray(W_v, dtype=np.float32)
    W_o = np.asarray(W_o, dtype=np.float32)

    nc = get_program()
    in_maps = make_in_maps(x, W_q, W_k, W_v, W_o)
    res = run_bass_kernel_spmd(
        nc,
        in_maps,
        list(range(NCORES)),
        trace=_trace,
        trace_cores=_trace_cores,
    )
    y = np.zeros((B, S, E), np.float32)
    for c in range(NCORES):
        y[c // 4] += res.results[c]["y"].astype(np.float32)
    if _trace:
        return y, res
    return y
